# revision 1
# baseline (speedup 1.0000x reference)
"""COLoRALinear fused kernel for 8 trn2 NeuronCores (Bass/Tile).

Problem: out = x@W.T + b + cw*2*(x@sA.T)@sB.T + (1-cw)*2*sum_t r[b,t]*(x@tA[t].T)@tB[t].T
with routing r = softmax(mean_s(x) @ emb.T), cw = sigmoid(collab_weight).

Sharding: core i -> batch element p=i//2 (2048 tokens), DOUT half h=i%2
(2048 cols). Each core holds its full batch element, so routing is local;
no collectives.

Device plan per core:
  - preload x^T as bf16, SBUF-resident ([128, 32, 2048], 128KB/part)
  - phase A: hid^T[80, 2048] = A_cat @ x^T  (A_cat rows: 8 shared + 64 task
    + 8 task_emb), evict rows 0..71 to bf16, free-reduce rows 72..79 into
    routing logits
  - routing: softmax on one partition, build svec[73] (cw2 / routing-scaled
    / 1.0-for-bias), scale B_cat rows -> bf16
  - main loop: 16 dout-tiles of 128; W^T d-tile streamed fp32->bf16 and
    used as the stationary operand, x^T slices as the moving operand
    (N=512); per d-tile: 4 PSUM banks accumulate 32 k matmuls + 1 LoRA
    down-proj matmul each; evict fp32; store [dout, tok] (host transposes).

Measured (reps-in-NEFF diff): ~408 us/core-iteration, rel err 1.65e-3.
W is staged per k-tile (64KB DMAs) in a dedicated 4-buf pool so the first
d-tiles' matmul chains chase the x preload stream instead of waiting for
it. Deeper W prefetch (wch bufs=3: 574 us) regressed — early W DMAs steal
HBM bandwidth from the startup-critical x preload.
"""
import numpy as np
from contextlib import ExitStack

import concourse.bass as bass
import concourse.tile as tile
from concourse import mybir
from concourse.bass_utils import run_bass_kernel_spmd
from concourse.vector_clock import ScopedClock

B, S, DIN, DOUT, R, T = 4, 2048, 4096, 4096, 8, 8
SCALING = 2.0
N_CORES = 8
P = 128
KT = DIN // P            # 32 k-tiles
S_CORE = S               # tokens per core (one batch element)
N_CORE = DOUT // 2       # dout columns per core
NCH = 256                # n-chunk width
NNC = N_CORE // NCH      # 8 n-chunks
MT = S_CORE // P         # 16 m-tiles
AROWS = 80               # 8 shared + 64 task + 8 emb rows in A_cat
HID = 73                 # 72 lora rows + ones(bias) row
F32 = mybir.dt.float32
BF16 = mybir.dt.bfloat16


class _DrainSplitTileContext(tile.TileContext):
    """Walrus in this container rejects a Drain carrying >1 sem wait (the
    CTRL_NO encoding has one TPB_EVENTS wait slot). Split the exit drain's
    waits across a chain of single-wait drains."""

    def _drain_and_barrier(self, tick_clock, wait_clock):
        drain_inst = self.nc.sync.drain()
        wait_clock.add_sem_waits(
            drain_inst.ins, ScopedClock({None: tick_clock.global_clock})
        )
        si = drain_inst.ins.sync_info
        if si is not None and len(si.on_wait) > 1:
            waits = list(si.on_wait)
            drain_inst.ins.sync_info = mybir.SyncInfo(
                on_wait=[waits[0]], on_update=list(si.on_update)
            )
            for w in waits[1:]:
                extra = self.nc.sync.drain()
                extra.ins.sync_info = mybir.SyncInfo(on_wait=[w], on_update=[])

        self.nc.all_engine_barrier()
        assert self.sems is not None
        popped = self.nc._tile_sem_poison_stack.pop()
        assert popped is self._sem_poison
        self.nc.clear_and_free_semaphores(list(self.sems.allocated().values()))
        self.nc.all_engine_barrier()


_wsplit_counter = [0]


def _split_multi_waits(nc):
    """Walrus here lowers DMA/CTRL instructions with a single TPB_EVENTS wait
    slot and rejects >1 sem wait. Hoist extra waits onto same-engine NoOps
    inserted immediately before the offending instruction (engine program
    order makes this semantics-preserving)."""
    for f in nc.m.functions:
        for blk in f.blocks:
            insts = blk.instructions
            out = []
            changed = False
            for inst in insts:
                si = inst.sync_info
                if si is not None and len(si.on_wait) > 1:
                    waits = list(si.on_wait)
                    for w in waits[:-1]:
                        _wsplit_counter[0] += 1
                        nop = mybir.InstNoOp(name=f"I-wsplit-{_wsplit_counter[0]}")
                        nop.engine = inst.engine
                        nop.sync_info = mybir.SyncInfo(on_wait=[w], on_update=[])
                        out.append(nop)
                    inst.sync_info = mybir.SyncInfo(
                        on_wait=[waits[-1]], on_update=list(si.on_update)
                    )
                    changed = True
                out.append(inst)
            if changed:
                blk.instructions = out


def build_nc(reps: int = 1):
    nc = bass.Bass(trn_type="TRN2", target_bir_lowering=False)
    xt = nc.dram_tensor("xt", [DIN, S_CORE], F32, kind="ExternalInput").ap()
    wt = nc.dram_tensor("wt", [DIN, N_CORE], F32, kind="ExternalInput").ap()
    act = nc.dram_tensor("act", [KT, P, AROWS], F32, kind="ExternalInput").ap()
    bcat = nc.dram_tensor("bcat", [HID, N_CORE], F32, kind="ExternalInput").ap()
    cw = nc.dram_tensor("cw", [1, 1], F32, kind="ExternalInput").ap()
    # output stored [dout, tok]; host assembly transposes back
    out = nc.dram_tensor("out", [N_CORE, S_CORE], F32, kind="ExternalOutput").ap()

    xt_r = xt.rearrange("(kt p) t -> p kt t", p=P)
    wt_r = wt.rearrange("(kt p) n -> p kt n", p=P)

    with _DrainSplitTileContext(nc) as tc, ExitStack() as ctx:
        xres_p = ctx.enter_context(tc.tile_pool(name="xres", bufs=1))
        wch_p = ctx.enter_context(tc.tile_pool(name="wch", bufs=2))
        stage_p = ctx.enter_context(tc.tile_pool(name="stage", bufs=2))
        wstage_p = ctx.enter_context(tc.tile_pool(name="wstage", bufs=4))
        abf_p = ctx.enter_context(tc.tile_pool(name="abf", bufs=1))
        small_p = ctx.enter_context(tc.tile_pool(name="small", bufs=1))
        evict_p = ctx.enter_context(tc.tile_pool(name="evict", bufs=3))
        psb_p = ctx.enter_context(tc.tile_pool(name="psb", bufs=7, space="PSUM"))
        pss_p = ctx.enter_context(tc.tile_pool(name="pss", bufs=1, space="PSUM"))

        for _rep in range(reps):
            # ---- constants / small preloads ----
            a_bf = abf_p.tile([P, KT, AROWS], BF16)
            act_r = act.rearrange("kt p c -> p kt c")
            for half in range(2):
                a_st = stage_p.tile([P, KT // 2, AROWS], F32, tag="stage")
                ks = slice(half * KT // 2, (half + 1) * KT // 2)
                nc.sync.dma_start(out=a_st[:], in_=act_r[:, ks, :])
                nc.vector.tensor_copy(out=a_bf[:, ks, :], in_=a_st[:])

            bmat = small_p.tile([HID, N_CORE], F32)
            nc.sync.dma_start(out=bmat[:], in_=bcat)

            cwt = small_p.tile([1, 1], F32)
            nc.sync.dma_start(out=cwt[:], in_=cw)
            sig = small_p.tile([1, 1], F32)
            nc.scalar.activation(
                out=sig[:], in_=cwt[:], func=mybir.ActivationFunctionType.Sigmoid
            )
            cw2 = small_p.tile([1, 1], F32)
            nc.vector.tensor_scalar_mul(cw2[:], sig[:], SCALING)
            tsc = small_p.tile([1, 1], F32)  # (1 - sigmoid) * SCALING
            nc.vector.tensor_scalar(
                out=tsc[:], in0=sig[:], scalar1=-SCALING, scalar2=SCALING,
                op0=mybir.AluOpType.mult, op1=mybir.AluOpType.add,
            )

            # ---- x preload (fp32 -> bf16, SBUF resident) ----
            xres = xres_p.tile([P, KT, S_CORE], BF16)
            for kt in range(KT):
                xs = stage_p.tile([P, S_CORE], F32, tag="stage")
                nc.sync.dma_start(out=xs[:], in_=xt_r[:, kt, :])
                nc.vector.tensor_copy(out=xres[:, kt, :], in_=xs[:])

            # ---- phase A: hid^T = A_cat @ x^T ----
            hid = small_p.tile([HID, S_CORE], BF16)
            hacc = small_p.tile([AROWS, 1], F32)     # free-reduced phase-A rows
            hpart = small_p.tile([AROWS, 4], F32)
            for c in range(4):
                ph = pss_p.tile([AROWS, 512], F32, tag="pss")
                for kt in range(KT):
                    nc.tensor.matmul(
                        ph[:], lhsT=a_bf[:, kt, :], rhs=xres[:, kt, c * 512:(c + 1) * 512],
                        start=(kt == 0), stop=(kt == KT - 1),
                    )
                nc.vector.tensor_copy(out=hid[0:72, c * 512:(c + 1) * 512], in_=ph[0:72, :])
                nc.vector.tensor_reduce(
                    out=hpart[:, c:c + 1], in_=ph[:], axis=mybir.AxisListType.X,
                    op=mybir.AluOpType.add,
                )
            ones_s = small_p.tile([1, P], BF16)
            nc.vector.memset(ones_s[:], 1.0)
            for m in range(MT):
                nc.sync.dma_start(out=hid[72:73, m * P:(m + 1) * P], in_=ones_s[:])
            nc.vector.tensor_reduce(
                out=hacc[:], in_=hpart[:], axis=mybir.AxisListType.X,
                op=mybir.AluOpType.add,
            )

            # ---- routing ----
            l_row = small_p.tile([1, 8], F32)
            nc.sync.dma_start(out=l_row[:], in_=hacc[72:80, 0:1])  # partition->free
            e_row = small_p.tile([1, 8], F32)
            nc.scalar.activation(
                out=e_row[:], in_=l_row[:], func=mybir.ActivationFunctionType.Exp,
                scale=1.0 / S,
            )
            ssum = small_p.tile([1, 1], F32)
            nc.vector.tensor_reduce(
                out=ssum[:], in_=e_row[:], axis=mybir.AxisListType.X,
                op=mybir.AluOpType.add,
            )
            rec = small_p.tile([1, 1], F32)
            nc.vector.reciprocal(out=rec[:], in_=ssum[:])
            comb = small_p.tile([1, 1], F32)  # (1/sum) * (1-cw)*SCALING
            nc.vector.tensor_tensor(
                out=comb[:], in0=rec[:], in1=tsc[:], op=mybir.AluOpType.mult
            )
            ones8 = small_p.tile([1, 8], F32)
            nc.vector.memset(ones8[:], 1.0)
            svec_f = small_p.tile([1, HID], F32)
            nc.vector.tensor_scalar(
                out=svec_f[0:1, 0:8], in0=ones8[:], scalar1=cw2[:], scalar2=None,
                op0=mybir.AluOpType.mult,
            )
            for t in range(T):
                nc.vector.tensor_scalar(
                    out=svec_f[0:1, 8 + 8 * t:16 + 8 * t], in0=ones8[:],
                    scalar1=e_row[0:1, t:t + 1], scalar2=comb[:],
                    op0=mybir.AluOpType.mult, op1=mybir.AluOpType.mult,
                )
            nc.vector.memset(svec_f[0:1, 72:73], 1.0)
            svec = small_p.tile([HID, 1], F32)
            nc.sync.dma_start(out=svec[:], in_=svec_f[:])  # free->partition
            bbf = small_p.tile([HID, N_CORE], BF16)
            nc.vector.tensor_scalar(
                out=bbf[:], in0=bmat[:], scalar1=svec[:], scalar2=None,
                op0=mybir.AluOpType.mult,
            )

            # ---- main loop: base matmul + fused down-proj ----
            # W^T d-tile is the stationary operand, x^T the moving one
            # (N=512); PSUM tiles come out [dout, tok].
            TC = 4  # token chunks of 512
            for d in range(N_CORE // P):
                wch = wch_p.tile([P, KT, P], BF16)
                for kt in range(KT):
                    ws = wstage_p.tile([P, P], F32)
                    nc.sync.dma_start(
                        out=ws[:], in_=wt_r[:, kt, d * P:(d + 1) * P]
                    )
                    nc.vector.tensor_copy(out=wch[:, kt, :], in_=ws[:])
                pss = [
                    psb_p.tile([P, 512], F32, tag="ps", name=f"ps_{_rep}_{d}_{i}")
                    for i in range(TC)
                ]
                for kt in range(KT):
                    for tcI in range(TC):
                        nc.tensor.matmul(
                            pss[tcI][:], lhsT=wch[:, kt, :],
                            rhs=xres[:, kt, tcI * 512:(tcI + 1) * 512],
                            start=(kt == 0), stop=False,
                        )
                for tcI in range(TC):
                    nc.tensor.matmul(
                        pss[tcI][:], lhsT=bbf[:, d * P:(d + 1) * P],
                        rhs=hid[:, tcI * 512:(tcI + 1) * 512],
                        start=False, stop=True,
                    )
                    ev = evict_p.tile([P, 512], F32)
                    nc.scalar.activation(
                        out=ev[:], in_=pss[tcI][:],
                        func=mybir.ActivationFunctionType.Copy,
                    )
                    nc.scalar.dma_start(
                        out=out[d * P:(d + 1) * P, tcI * 512:(tcI + 1) * 512],
                        in_=ev[:],
                    )
    _split_multi_waits(nc)
    return nc


def prep_inputs(x, W, b, shared_A, shared_B, task_A, task_B, task_emb, collab_weight):
    """Host-side sharding/layout prep. Pure layout: slice/transpose/concat."""
    x = np.asarray(x, dtype=np.float32)
    W = np.asarray(W, dtype=np.float32)
    b = np.asarray(b, dtype=np.float32)
    a_cat = np.concatenate(
        [np.asarray(shared_A), np.asarray(task_A).reshape(T * R, DIN),
         np.asarray(task_emb)], axis=0
    ).astype(np.float32)                                   # [80, DIN]
    act = np.ascontiguousarray(a_cat.T.reshape(KT, P, AROWS))
    cwv = np.asarray(collab_weight, dtype=np.float32).reshape(1, 1)

    xt = [np.ascontiguousarray(x[p].T) for p in range(B)]  # [DIN, S] each
    wt, bc = [], []
    for h in range(2):
        cols = slice(h * N_CORE, (h + 1) * N_CORE)
        wt.append(np.ascontiguousarray(W[cols, :].T))      # [DIN, N_CORE]
        bcat = np.empty((HID, N_CORE), dtype=np.float32)
        bcat[0:8] = np.asarray(shared_B)[cols, :].T
        bcat[8:72] = np.asarray(task_B)[:, cols, :].transpose(0, 2, 1).reshape(
            T * R, N_CORE
        )
        bcat[72] = b[cols]
        bc.append(bcat)

    in_maps = []
    for i in range(N_CORES):
        p, h = i // 2, i % 2
        in_maps.append(
            {"xt": xt[p], "wt": wt[h], "act": act, "bcat": bc[h], "cw": cwv}
        )
    return in_maps


def assemble(results):
    out = np.empty((B, S, DOUT), dtype=np.float32)
    for i in range(N_CORES):
        p, h = i // 2, i % 2
        out[p, :, h * N_CORE:(h + 1) * N_CORE] = results[i]["out"].T
    return out


_NC_CACHE = None


def kernel(**inputs) -> np.ndarray:
    global _NC_CACHE
    if _NC_CACHE is None:
        _NC_CACHE = build_nc()
    in_maps = prep_inputs(**inputs)
    res = run_bass_kernel_spmd(_NC_CACHE, in_maps, core_ids=list(range(N_CORES)))
    return assemble(res.results)



# revision 2
# speedup vs baseline: 2.7578x; 2.7578x over previous
"""COLoRALinear fused kernel for 8 trn2 NeuronCores — hybrid fp8/bf16.

Problem: out = x@W.T + b + cw*2*(x@sA.T)@sB.T + (1-cw)*2*sum_t r[b,t]*(x@tA[t].T)@tB[t].T
with routing r = softmax(mean_s(x) @ emb.T), cw = sigmoid(collab_weight).

Sharding: core i -> batch element p=i//2 (2048 tokens), DOUT half h=i%2
(2048 cols). Routing is local per core; no collectives.

Numerics: split the DIN=4096 contraction into FT8 k-tiles (of 128)
computed in fp8-e4m3 DoubleRow perf mode (2 fp8 contraction slots per PE
cell per cycle ~ 2.1x bf16 FLOPs) and the remaining 32-FT8 k-tiles in
bf16. fp8 quantization noise from the full contraction is ~2.5e-2 rel;
the alpha=FT8/32 fraction scales it by sqrt(alpha). Operands are
pre-scaled on the host (x*16, W*512) so fp8 and bf16 partial products
accumulate in PSUM at one scale; evictions rescale by 2^-13.

Device plan per core:
  - preload x fp8-part [128, FT8/2, 2, S] + bf16-part [128, 32-FT8, S],
    SBUF resident
  - phase A: hid = A @ x over 4 token chunks (fp8-DR + bf16 matmuls
    mirroring the x split), evict rows 0..71 bf16, free-reduce rows
    72..79 into routing logits
  - routing softmax on one partition; svec scales B_cat rows by
    8192*(cw-or-routing); down-proj stays bf16
  - main loop: 16 dout-tiles; W d-tiles streamed in both dtypes; per
    (d, chunk-of-512): FT8/2 DoubleRow + (32-FT8) bf16 + 1 down-proj
    matmul into one PSUM bank; evict bf16 [dout, tok]
"""
import numpy as np
import ml_dtypes
from contextlib import ExitStack

import concourse.bass as bass
import concourse.tile as tile
from concourse import mybir
from concourse.bass_utils import run_bass_kernel_spmd
from concourse.vector_clock import ScopedClock

B, S, DIN, DOUT, R, T = 4, 2048, 4096, 4096, 8, 8
SCALING = 2.0
N_CORES = 8
P = 128
KT = DIN // P            # 32 k-tiles
FT8 = 16                 # k-tiles done in fp8 (even); alpha = FT8/32
KP8 = FT8 // 2           # fp8 k-pairs
KTB = KT - FT8           # bf16 k-tiles
S_CORE = S
N_CORE = DOUT // 2
CW = 512                 # token chunk width
NCH = S_CORE // CW       # 4 chunks
DT = N_CORE // P         # 16 dout tiles
AROWS = 80
HID = 73
SX = 16.0
SW = 512.0
SXW = SX * SW            # 8192
F32 = mybir.dt.float32
BF16 = mybir.dt.bfloat16
FP8 = mybir.dt.float8e4
E4 = ml_dtypes.float8_e4m3
BF = ml_dtypes.bfloat16
DR = mybir.MatmulPerfMode.DoubleRow
ORDER = "bf16_first"  # or "dr_first"


class _DrainSplitTileContext(tile.TileContext):
    """Walrus in this container rejects a Drain carrying >1 sem wait (the
    CTRL_NO encoding has one TPB_EVENTS wait slot). Split the exit drain's
    waits across a chain of single-wait drains."""

    def _drain_and_barrier(self, tick_clock, wait_clock):
        drain_inst = self.nc.sync.drain()
        wait_clock.add_sem_waits(
            drain_inst.ins, ScopedClock({None: tick_clock.global_clock})
        )
        si = drain_inst.ins.sync_info
        if si is not None and len(si.on_wait) > 1:
            waits = list(si.on_wait)
            drain_inst.ins.sync_info = mybir.SyncInfo(
                on_wait=[waits[0]], on_update=list(si.on_update)
            )
            for w in waits[1:]:
                extra = self.nc.sync.drain()
                extra.ins.sync_info = mybir.SyncInfo(on_wait=[w], on_update=[])

        self.nc.all_engine_barrier()
        assert self.sems is not None
        popped = self.nc._tile_sem_poison_stack.pop()
        assert popped is self._sem_poison
        self.nc.clear_and_free_semaphores(list(self.sems.allocated().values()))
        self.nc.all_engine_barrier()


_wsplit_counter = [0]


def _split_multi_waits(nc):
    """Walrus here lowers DMA/CTRL instructions with a single TPB_EVENTS wait
    slot and rejects >1 sem wait. Hoist extra waits onto same-engine NoOps
    inserted immediately before the offending instruction (engine program
    order makes this semantics-preserving)."""
    for f in nc.m.functions:
        for blk in f.blocks:
            insts = blk.instructions
            out = []
            changed = False
            for inst in insts:
                si = inst.sync_info
                if si is not None and len(si.on_wait) > 1:
                    waits = list(si.on_wait)
                    for w in waits[:-1]:
                        _wsplit_counter[0] += 1
                        nop = mybir.InstNoOp(name=f"I-wsplit-{_wsplit_counter[0]}")
                        nop.engine = inst.engine
                        nop.sync_info = mybir.SyncInfo(on_wait=[w], on_update=[])
                        out.append(nop)
                    inst.sync_info = mybir.SyncInfo(
                        on_wait=[waits[-1]], on_update=list(si.on_update)
                    )
                    changed = True
                out.append(inst)
            if changed:
                blk.instructions = out
    return nc


def _emit_rep(nc, tc, pools, dram, _rep):
    (xres_p, wch_p, small_p, evict_p, psb_p, pss_p) = pools
    (x8_d, xb_d, w8_d, wb_d, a8_d, ab_d, bcat, cw, out) = dram

    # ---- small preloads ----
    if KP8:
        a8 = small_p.tile([P, KP8, 2, AROWS], FP8, tag="a8", name=f"a8_{_rep}")
        nc.sync.dma_start(out=a8[:], in_=a8_d)
    ab = small_p.tile([P, KTB, AROWS], BF16, tag="ab", name=f"ab_{_rep}")
    nc.sync.dma_start(out=ab[:], in_=ab_d)
    bmat = small_p.tile([HID, N_CORE], F32, tag="bm", name=f"bm_{_rep}")
    nc.sync.dma_start(out=bmat[:], in_=bcat)
    cwt = small_p.tile([1, 1], F32, tag="cwt", name=f"cw_{_rep}")
    nc.sync.dma_start(out=cwt[:], in_=cw)
    sig = small_p.tile([1, 1], F32, tag="sig", name=f"sig_{_rep}")
    nc.scalar.activation(
        out=sig[:], in_=cwt[:], func=mybir.ActivationFunctionType.Sigmoid
    )
    cw2 = small_p.tile([1, 1], F32, tag="cw2", name=f"cw2_{_rep}")
    nc.vector.tensor_scalar_mul(cw2[:], sig[:], SCALING * SXW)
    tsc = small_p.tile([1, 1], F32, tag="tsc", name=f"tsc_{_rep}")
    nc.vector.tensor_scalar(
        out=tsc[:], in0=sig[:], scalar1=-SCALING * SXW, scalar2=SCALING * SXW,
        op0=mybir.AluOpType.mult, op1=mybir.AluOpType.add,
    )

    # ---- x preload (SBUF resident; fp8 part then bf16 part) ----
    if KP8:
        x8 = xres_p.tile([P, KP8, 2, S_CORE], FP8, tag="x8", name=f"x8_{_rep}")
        for kp in range(KP8):
            nc.sync.dma_start(out=x8[:, kp, :, :], in_=x8_d[:, kp, :, :])
    xb = xres_p.tile([P, KTB, S_CORE], BF16, tag="xb", name=f"xb_{_rep}", bufs=2)
    for kt in range(KTB):
        nc.sync.dma_start(out=xb[:, kt, :], in_=xb_d[:, kt, :])

    # ---- phase A ----
    hid = small_p.tile([HID, S_CORE], BF16, tag="hid", name=f"hid_{_rep}", bufs=2)
    hacc = small_p.tile([AROWS, 1], F32, tag="hacc", name=f"hacc_{_rep}")
    hpart = small_p.tile([AROWS, NCH], F32, tag="hp", name=f"hp_{_rep}")
    for c in range(NCH):
        cs = slice(c * CW, (c + 1) * CW)
        ph = pss_p.tile([AROWS, CW], F32, tag="pss", name=f"ph_{_rep}_{c}")
        ops = ([("b", kt) for kt in range(KTB)] + [("8", kp) for kp in range(KP8)]
               if ORDER == "bf16_first" else
               [("8", kp) for kp in range(KP8)] + [("b", kt) for kt in range(KTB)])
        for i, (kind, j) in enumerate(ops):
            if kind == "b":
                nc.tensor.matmul(
                    ph[:], lhsT=ab[:, j, :], rhs=xb[:, j, cs],
                    start=(i == 0), stop=(i == len(ops) - 1),
                    skip_group_check=True,
                )
            else:
                nc.tensor.matmul(
                    ph[:], lhsT=a8[:, j, :, :], rhs=x8[:, j, :, cs],
                    start=(i == 0), stop=(i == len(ops) - 1), perf_mode=DR,
                    skip_group_check=True,
                )
        nc.scalar.activation(
            out=hid[0:72, cs], in_=ph[0:72, :],
            func=mybir.ActivationFunctionType.Copy, scale=1.0 / SXW,
        )
        nc.vector.tensor_reduce(
            out=hpart[:, c:c + 1], in_=ph[:], axis=mybir.AxisListType.X,
            op=mybir.AluOpType.add,
        )
    ones_s = small_p.tile([1, S_CORE], BF16, tag="on", name=f"on_{_rep}")
    nc.vector.memset(ones_s[:], 1.0)
    nc.sync.dma_start(out=hid[72:73, :], in_=ones_s[:])
    nc.vector.tensor_reduce(
        out=hacc[:], in_=hpart[:], axis=mybir.AxisListType.X,
        op=mybir.AluOpType.add,
    )

    # ---- routing ----
    l_row = small_p.tile([1, 8], F32, tag="lr", name=f"lr_{_rep}")
    nc.sync.dma_start(out=l_row[:], in_=hacc[72:80, 0:1])  # partition->free
    e_row = small_p.tile([1, 8], F32, tag="er", name=f"er_{_rep}")
    nc.scalar.activation(
        out=e_row[:], in_=l_row[:], func=mybir.ActivationFunctionType.Exp,
        scale=1.0 / (S * SXW),
    )
    ssum = small_p.tile([1, 1], F32, tag="ss", name=f"ss_{_rep}")
    nc.vector.tensor_reduce(
        out=ssum[:], in_=e_row[:], axis=mybir.AxisListType.X,
        op=mybir.AluOpType.add,
    )
    rec = small_p.tile([1, 1], F32, tag="rc", name=f"rc_{_rep}")
    nc.vector.reciprocal(out=rec[:], in_=ssum[:])
    comb = small_p.tile([1, 1], F32, tag="cb", name=f"cb_{_rep}")
    nc.vector.tensor_tensor(
        out=comb[:], in0=rec[:], in1=tsc[:], op=mybir.AluOpType.mult
    )
    ones8 = small_p.tile([1, 8], F32, tag="o8", name=f"o8_{_rep}")
    nc.vector.memset(ones8[:], 1.0)
    svec_f = small_p.tile([1, HID], F32, tag="sv", name=f"sv_{_rep}")
    nc.vector.tensor_scalar(
        out=svec_f[0:1, 0:8], in0=ones8[:], scalar1=cw2[:], scalar2=None,
        op0=mybir.AluOpType.mult,
    )
    for t in range(T):
        nc.vector.tensor_scalar(
            out=svec_f[0:1, 8 + 8 * t:16 + 8 * t], in0=ones8[:],
            scalar1=e_row[0:1, t:t + 1], scalar2=comb[:],
            op0=mybir.AluOpType.mult, op1=mybir.AluOpType.mult,
        )
    nc.vector.memset(svec_f[0:1, 72:73], SXW)
    svec = small_p.tile([HID, 1], F32, tag="svp", name=f"svp_{_rep}")
    nc.sync.dma_start(out=svec[:], in_=svec_f[:])  # free->partition
    bbf = small_p.tile([HID, N_CORE], BF16, tag="bbf", name=f"bbf_{_rep}", bufs=2)
    nc.vector.tensor_scalar(
        out=bbf[:], in0=bmat[:], scalar1=svec[:], scalar2=None,
        op0=mybir.AluOpType.mult,
    )

    # ---- main loop ----
    for d in range(DT):
        if KP8:
            w8 = wch_p.tile([P, KP8, 2, P], FP8, tag="w8", name=f"w8_{_rep}_{d}")
            nc.sync.dma_start(out=w8[:], in_=w8_d[d])
        wb = wch_p.tile([P, KTB, P], BF16, tag="wb", name=f"wb_{_rep}_{d}")
        nc.sync.dma_start(out=wb[:], in_=wb_d[d])
        for c in range(NCH):
            cs = slice(c * CW, (c + 1) * CW)
            ps = psb_p.tile([P, CW], F32, tag="ps", name=f"ps_{_rep}_{d}_{c}")
            ops = ([("b", kt) for kt in range(KTB)] + [("8", kp) for kp in range(KP8)]
                   if ORDER == "bf16_first" else
                   [("8", kp) for kp in range(KP8)] + [("b", kt) for kt in range(KTB)])
            for i, (kind, j) in enumerate(ops):
                if kind == "b":
                    nc.tensor.matmul(
                        ps[:], lhsT=wb[:, j, :], rhs=xb[:, j, cs],
                        start=(i == 0), stop=False,
                        skip_group_check=True,
                    )
                else:
                    nc.tensor.matmul(
                        ps[:], lhsT=w8[:, j, :, :], rhs=x8[:, j, :, cs],
                        start=(i == 0), stop=False, perf_mode=DR,
                        skip_group_check=True,
                    )
            nc.tensor.matmul(
                ps[:], lhsT=bbf[:, d * P:(d + 1) * P], rhs=hid[:, cs],
                start=False, stop=True, skip_group_check=True,
            )
            ev = evict_p.tile([P, CW], BF16, tag="ev")
            nc.scalar.activation(
                out=ev[:], in_=ps[:],
                func=mybir.ActivationFunctionType.Copy, scale=1.0 / SXW,
            )
            nc.scalar.dma_start(out=out[d * P:(d + 1) * P, cs], in_=ev[:])


def build_nc(reps: int = 1, loop: int = 0):
    """reps: python-unrolled repetitions. loop: if >0, wrap the body in a
    hardware For_i loop with `loop` iterations (timing builds)."""
    nc = bass.Bass(trn_type="TRN2", target_bir_lowering=False)
    x8_d = nc.dram_tensor("x8", [P, max(KP8, 1), 2, S_CORE], FP8,
                          kind="ExternalInput").ap()
    xb_d = nc.dram_tensor("xb", [P, KTB, S_CORE], BF16, kind="ExternalInput").ap()
    w8_d = nc.dram_tensor("w8", [DT, P, max(KP8, 1), 2, P], FP8,
                          kind="ExternalInput").ap()
    wb_d = nc.dram_tensor("wb", [DT, P, KTB, P], BF16, kind="ExternalInput").ap()
    a8_d = nc.dram_tensor("a8", [P, max(KP8, 1), 2, AROWS], FP8,
                          kind="ExternalInput").ap()
    ab_d = nc.dram_tensor("ab", [P, KTB, AROWS], BF16, kind="ExternalInput").ap()
    bcat = nc.dram_tensor("bcat", [HID, N_CORE], F32, kind="ExternalInput").ap()
    cw = nc.dram_tensor("cw", [1, 1], F32, kind="ExternalInput").ap()
    out = nc.dram_tensor("out", [N_CORE, S_CORE], BF16, kind="ExternalOutput").ap()
    dram = (x8_d, xb_d, w8_d, wb_d, a8_d, ab_d, bcat, cw, out)

    with _DrainSplitTileContext(nc) as tc, ExitStack() as ctx:
        pools = (
            ctx.enter_context(tc.tile_pool(name="xres", bufs=1)),
            ctx.enter_context(tc.tile_pool(name="wch", bufs=2)),
            ctx.enter_context(tc.tile_pool(name="small", bufs=1)),
            ctx.enter_context(tc.tile_pool(name="evict", bufs=3)),
            ctx.enter_context(tc.tile_pool(name="psb", bufs=5, space="PSUM")),
            ctx.enter_context(tc.tile_pool(name="pss", bufs=2, space="PSUM")),
        )
        if loop:
            # two unrolled reps inside the hw loop so bufs=2 tags alternate
            # slots across iterations (matching the unrolled build's overlap)
            with tc.For_i(0, loop, 1):
                _emit_rep(nc, tc, pools, dram, 0)
                _emit_rep(nc, tc, pools, dram, 1)
        else:
            for _rep in range(reps):
                _emit_rep(nc, tc, pools, dram, _rep)
    _split_multi_waits(nc)
    return nc


def _pack_pairs(a):
    """[DIN_sub, F] fp8 (already scaled) -> [P, KP, 2, F], k = kp*256+s*128+p."""
    f = a.shape[1]
    kp = a.shape[0] // 256
    return np.ascontiguousarray(a.reshape(kp, 2, P, f).transpose(2, 0, 1, 3))


def _pack_tiles(a):
    """[DIN_sub, F] bf16 -> [P, KT_sub, F]."""
    f = a.shape[1]
    kt = a.shape[0] // P
    return np.ascontiguousarray(a.reshape(kt, P, f).transpose(1, 0, 2))


def prep_inputs(x, W, b, shared_A, shared_B, task_A, task_B, task_emb, collab_weight):
    """Host-side sharding/layout/quantization prep."""
    K8 = FT8 * P  # first K8 of DIN in fp8
    x = np.asarray(x, dtype=np.float32)
    W = np.asarray(W, dtype=np.float32)
    b = np.asarray(b, dtype=np.float32)
    a_cat = np.concatenate(
        [np.asarray(shared_A), np.asarray(task_A).reshape(T * R, DIN),
         np.asarray(task_emb)], axis=0
    ).astype(np.float32).T * SW                            # [DIN, 80], scaled
    if FT8:
        a8 = _pack_pairs(np.clip(a_cat[:K8], -240, 240).astype(E4))
    else:
        a8 = np.zeros((P, 1, 2, AROWS), E4)
    ab = _pack_tiles(a_cat[K8:].astype(BF))
    cwv = np.asarray(collab_weight, dtype=np.float32).reshape(1, 1)

    x8s, xbs = [], []
    for p in range(B):
        xt = np.ascontiguousarray(x[p].T) * SX             # [DIN, S], scaled
        if FT8:
            x8s.append(_pack_pairs(np.clip(xt[:K8], -240, 240).astype(E4)))
        else:
            x8s.append(np.zeros((P, 1, 2, S_CORE), E4))
        xbs.append(_pack_tiles(xt[K8:].astype(BF)))

    w8s, wbs, bc = [], [], []
    for h in range(2):
        cols = slice(h * N_CORE, (h + 1) * N_CORE)
        wt = np.ascontiguousarray(W[cols, :].T) * SW       # [DIN, N_CORE], scaled
        if FT8:
            p8 = _pack_pairs(np.clip(wt[:K8], -240, 240).astype(E4))
            w8s.append(np.ascontiguousarray(
                p8.reshape(P, KP8, 2, DT, P).transpose(3, 0, 1, 2, 4)))
        else:
            w8s.append(np.zeros((DT, P, 1, 2, P), E4))
        pb = _pack_tiles(wt[K8:].astype(BF))
        wbs.append(np.ascontiguousarray(
            pb.reshape(P, KTB, DT, P).transpose(2, 0, 1, 3)))
        bcat = np.empty((HID, N_CORE), dtype=np.float32)
        bcat[0:8] = np.asarray(shared_B)[cols, :].T
        bcat[8:72] = np.asarray(task_B)[:, cols, :].transpose(0, 2, 1).reshape(
            T * R, N_CORE
        )
        bcat[72] = b[cols]
        bc.append(bcat)

    in_maps = []
    for i in range(N_CORES):
        p, h = i // 2, i % 2
        in_maps.append(
            {"x8": x8s[p], "xb": xbs[p], "w8": w8s[h], "wb": wbs[h],
             "a8": a8, "ab": ab, "bcat": bc[h], "cw": cwv}
        )
    return in_maps


def assemble(results):
    out = np.empty((B, S, DOUT), dtype=np.float32)
    for i in range(N_CORES):
        p, h = i // 2, i % 2
        out[p, :, h * N_CORE:(h + 1) * N_CORE] = (
            results[i]["out"].astype(np.float32).T
        )
    return out


_NC_CACHE = None


def kernel(**inputs) -> np.ndarray:
    global _NC_CACHE
    if _NC_CACHE is None:
        _NC_CACHE = build_nc()
    in_maps = prep_inputs(**inputs)
    res = run_bass_kernel_spmd(_NC_CACHE, in_maps, core_ids=list(range(N_CORES)))
    return assemble(res.results)
